# revision 44
# baseline (speedup 1.0000x reference)
"""Trainium2 Bass kernel for nn_CustomMultiresLayer (B=2, D=1024, L=4096, FS=4).

Sharding (8 cores): core c -> batch beta=c//4, channel shard gamma=c%4
(256 channels = 2 half-slabs of 128). Phase A computes the depthwise
multires tree + gated combination for the core's 256 channels; the two
128-channel half-slabs are processed sequentially (h=0 fully first) so
the AllGather of half 0 overlaps half 1's tree. Phase B: channel mix as
bf16 matmuls (residual folded in via an identity matmul), partial LN
stats, AllReduce of [2,4096] stats, normalize, store.

Engine split in the tree, per level: a-chain convs on PE (bf16 diagonal
weight matmuls, fp32 PSUM, ACT copyout to bf16), b convs on DVE in bf16
(2x mode), sigmoid on ACT, gate-mul on DVE (bf16 2x), y accumulation on
GPSIMD in fp32.
"""

import numpy as np
import ml_dtypes

import concourse.bacc as bacc
import concourse.mybir as mybir
import concourse.tile as tile
from concourse.bass_utils import run_bass_kernel_spmd

F32 = mybir.dt.float32
BF16 = mybir.dt.bfloat16
AF = mybir.ActivationFunctionType
ALU = mybir.AluOpType

B, D, L = 2, 1024, 4096
FS, DEPTH = 4, 11
LN_EPS = 1e-5
NC = 8
CH = 256          # channels per core (2 half-slabs of 128)
WIN = 1536        # PSUM conv window (3 banks; 2 bufs + 2 pass1 banks = 8)
NMM = 512         # matmul moving-dim tile
GROUPS = [[0, 1, 2, 3], [4, 5, 6, 7]]

_CACHE = {}


def _emit_conv_pe(nc, cps, dst, src, diag, hi=L):
    """4-tap dilated causal depthwise conv via diagonal-weight matmuls on
    cols [0, hi). dst, src: SBUF bf16 [128, L]; diag: SBUF bf16
    [128, 4*128]. Tap-major emission within each PSUM window so identical
    stationary weights are consecutive."""
    dil = diag["dil"]
    dg = diag["t"]
    for w0 in range(0, hi, WIN):
        pp = cps.tile([128, WIN], F32, tag="cps", name="cps")
        nch = (min(hi, w0 + WIN) - w0) // NMM
        # per chunk: list of valid taps, to place start/stop flags
        valid = []
        for ci in range(nch):
            c0 = w0 + NMM * ci
            valid.append([k for k in (3, 2, 1, 0)
                          if max(0, (3 - k) * dil - c0) < NMM])
        for k in (3, 2, 1, 0):
            s = (3 - k) * dil
            for ci in range(nch):
                c0 = w0 + NMM * ci
                lo = max(0, s - c0)
                if lo >= NMM:
                    continue
                nc.tensor.matmul(
                    pp[:, NMM * ci + lo : NMM * (ci + 1)],
                    dg[:, 128 * k : 128 * (k + 1)],
                    src[:, c0 + lo - s : c0 + NMM - s],
                    start=(k == 3),
                    stop=(k == valid[ci][-1]),
                    skip_group_check=True,
                )
        nc.scalar.copy(dst[:, w0 : w0 + NMM * nch], pp[:, 0 : NMM * nch])


def _emit_conv_act_dve(nc, dst, src, h32, zb, dil):
    """4-tap conv: tap 3 (s=0) as a per-channel scaled copy on ACT (fp32
    scale AP), the remaining taps as DVE STT MACs (fp32 scalars — bf16
    scalar APs take a slower DVE path)."""
    nc.scalar.activation(dst[:], src[:], AF.Identity, bias=zb[:], scale=h32[:, 3:4])
    for k in (2, 1, 0):
        s = (3 - k) * dil
        nc.vector.scalar_tensor_tensor(
            dst[:, s:L], src[:, 0 : L - s], h32[:, k : k + 1], dst[:, s:L],
            ALU.mult, ALU.add,
        )


def _build_program():
    nc = bacc.Bacc("TRN2", target_bir_lowering=False, debug=False, num_devices=NC)

    xs16 = nc.dram_tensor("xs16", [CH, L], BF16, kind="ExternalInput").ap()
    h1s = nc.dram_tensor("h1s", [CH, FS], F32, kind="ExternalInput").ap()
    d0 = nc.dram_tensor("d0", [2, FS, 128, 128], BF16, kind="ExternalInput").ap()
    d1x2 = nc.dram_tensor("d1x2", [2, FS, 128, 128], BF16, kind="ExternalInput").ap()
    eye = nc.dram_tensor("eye", [128, 128], BF16, kind="ExternalInput").ap()
    wTs = nc.dram_tensor("wTs", [D, CH], BF16, kind="ExternalInput").ap()
    bmixs = nc.dram_tensor("bmixs", [CH, 1], F32, kind="ExternalInput").ap()
    gams = nc.dram_tensor("gams", [1, CH], BF16, kind="ExternalInput").ap()
    bets = nc.dram_tensor("bets", [1, CH], BF16, kind="ExternalInput").ap()
    og = nc.dram_tensor("og", [CH, L], F32, kind="ExternalOutput").ap()

    with tile.TileContext(nc) as tc:
        with (
            tc.tile_pool(name="dram", bufs=1, space="DRAM") as dram,
            tc.tile_pool(name="keep", bufs=1) as keep,
            tc.tile_pool(name="smalls", bufs=1) as smalls,
        ):
            y_loc = [dram.tile([128, L], BF16, name=f"yloc{h}") for h in range(2)]
            y_gat = [dram.tile([512, L], BF16, name=f"ygat{h}") for h in range(2)]
            st_loc = dram.tile([2, L], F32, name="stloc")
            st_glb = dram.tile([2, L], F32, name="stglb")
            st_fin = dram.tile([2, L], BF16, name="stfin")

            # x in bf16 — both a_0 for the tree and the mix residual
            x16 = [keep.tile([128, L], BF16, name=f"x16{h}") for h in range(2)]
            for h in range(2):
                nc.sync.dma_start(x16[h][:], xs16[128 * h : 128 * (h + 1), :])

            # phase-B constants (loaded early, tiny)
            wsb = keep.tile([128, 8 * CH], BF16, name="wsb")
            eyesb = smalls.tile([128, 128], BF16, name="eyesb")
            bsc = smalls.tile([128, 2], F32, name="bsc")
            grow = smalls.tile([1, CH], BF16, name="grow")
            brow = smalls.tile([1, CH], BF16, name="brow")
            ones16 = smalls.tile([128, 1], BF16, name="ones16")
            one_r = smalls.tile([1, NMM], BF16, name="oner")
            nc.vector.memset(ones16[:], 1.0)
            nc.vector.memset(one_r[:], 1.0)
            nc.sync.dma_start(eyesb[:], eye)
            nc.sync.dma_start(grow[:], gams)
            nc.sync.dma_start(brow[:], bets)
            for k in range(8):
                nc.sync.dma_start(
                    wsb[:, CH * k : CH * (k + 1)], wTs[128 * k : 128 * (k + 1), :]
                )
            for o in range(2):
                nc.sync.dma_start(bsc[:, o : o + 1], bmixs[128 * o : 128 * (o + 1), :])

            # ---------------- Phase A: multires tree ----------------
            h1c = [smalls.tile([128, FS], F32, name=f"h1c{h}") for h in range(2)]
            d0c = [smalls.tile([128, FS * 128], BF16, name=f"d0c{h}") for h in range(2)]
            d1c = [smalls.tile([128, FS * 128], BF16, name=f"d1c{h}") for h in range(2)]
            zb = smalls.tile([128, 1], F32, name="zb")
            nc.vector.memset(zb[:], 0.0)
            for h in range(2):
                nc.sync.dma_start(h1c[h][:], h1s[128 * h : 128 * (h + 1), :])
                for k in range(FS):
                    ks = slice(128 * k, 128 * (k + 1))
                    nc.sync.dma_start(d0c[h][:, ks], d0[h, k])
                    nc.sync.dma_start(d1c[h][:, ks], d1x2[h, k])

            # pass-1 mix resources (outer pools: distinct SBUF/PSUM, so
            # pass 1 can execute during the h=1 tree / AllGather window)
            yh0 = [keep.tile([128, 4 * (L // 2)], BF16, name=f"yh0{p}")
                   for p in range(2)]
            part16 = keep.tile([128, 2 * L], BF16, name="part16")

            with tc.tile_pool(name="ppa", bufs=2, space="PSUM") as ppa:
              with (
                tc.tile_pool(name="tree", bufs=1) as tp,
                tc.tile_pool(name="cpsum", bufs=2, space="PSUM") as cps,
              ):
                for h in range(2):
                    a_t = [tp.tile([128, L], BF16, tag=f"a{h}{i}", name=f"a{h}{i}")
                           for i in range(2)]
                    b_t = [tp.tile([128, L], BF16, tag=f"b{h}{i}", name=f"b{h}{i}")
                           for i in range(2)]
                    sg = tp.tile([128, L], BF16, tag=f"sg{h}", name=f"sg{h}")
                    gt = tp.tile([128, L], BF16, tag=f"gt{h}", name=f"gt{h}")
                    y_t = tp.tile([128, L], F32, tag=f"y{h}", name=f"y{h}")

                    for lvl in range(DEPTH):
                        dil = 1 << lvl
                        a_cur = x16[h] if lvl == 0 else a_t[lvl % 2]
                        a_nxt = a_t[(lvl + 1) % 2]
                        _emit_conv_pe(nc, cps, a_nxt, a_cur,
                                      {"t": d0c[h], "dil": dil})
                        if lvl == 0:
                            # b_0 with doubled h1 (folds the reused last
                            # gated term), on PE to dodge odd-offset DVE
                            _emit_conv_pe(nc, cps, b_t[0], a_cur,
                                          {"t": d1c[h], "dil": 1})
                        elif lvl < DEPTH - 1:
                            _emit_conv_act_dve(nc, b_t[lvl % 2], a_cur,
                                               h1c[h], zb, dil)
                        if lvl >= 1:
                            nc.scalar.activation(sg[:], a_nxt[:], AF.Sigmoid)
                            nc.vector.tensor_mul(gt[:], sg[:], b_t[(lvl + 1) % 2][:])
                            if lvl == 1:
                                nc.vector.tensor_copy(y_t[:], gt[:])
                            else:
                                nc.gpsimd.tensor_add(y_t[:], y_t[:], gt[:])

                    # sg is dead after the last gate — reuse it as the
                    # bf16 staging for the AllGather shard
                    nc.vector.tensor_copy(sg[:], y_t[:])
                    nc.sync.dma_start(y_loc[h][:, :], sg[:])
                    nc.gpsimd.collective_compute(
                        "AllGather",
                        ALU.bypass,
                        replica_groups=GROUPS,
                        ins=[y_loc[h].opt()],
                        outs=[y_gat[h].opt()],
                    )

              # ---- mix pass 1: kb 0-3 (needs only AllGather 0) ----
              for ph in range(2):
                  for kb in range(4):
                      nc.sync.dma_start(
                          yh0[ph][:, (L // 2) * kb : (L // 2) * (kb + 1)],
                          y_gat[0][128 * kb : 128 * (kb + 1),
                                   (L // 2) * ph : (L // 2) * (ph + 1)],
                      )
                  for nth in range(L // 2 // NMM):
                      for o in range(2):
                          pc = slice(L * o + (L // 2) * ph + NMM * nth,
                                     L * o + (L // 2) * ph + NMM * (nth + 1))
                          pmA = ppa.tile([128, NMM], F32, tag="pmA", name="pmA")
                          for kb in range(4):
                              nc.tensor.matmul(
                                  pmA[:],
                                  wsb[:, CH * kb + 128 * o :
                                      CH * kb + 128 * (o + 1)],
                                  yh0[ph][:, (L // 2) * kb + NMM * nth :
                                          (L // 2) * kb + NMM * (nth + 1)],
                                  start=(kb == 0),
                                  stop=(kb == 3),
                              )
                          nc.scalar.copy(part16[:, pc], pmA[:])

            # ---------------- Phase B: channel mix + LayerNorm ----------------
            with (
                tc.tile_pool(name="mix", bufs=1) as mx,
                tc.tile_pool(name="yld", bufs=2) as yld,
                tc.tile_pool(name="tiny", bufs=2) as tiny,
            ):
                z16 = mx.tile([128, 2 * L], BF16, name="z16")
                with (
                    tc.tile_pool(name="mmps", bufs=4, space="PSUM") as psmm,
                    tc.tile_pool(name="stps", bufs=2, space="PSUM") as psst,
                ):
                    for ph in range(2):
                        yhs = yld.tile([128, 4 * (L // 2)], BF16, tag="yhs", name="yhs")
                        for kb in range(4, 8):
                            nc.sync.dma_start(
                                yhs[:, (L // 2) * (kb - 4) : (L // 2) * (kb - 3)],
                                y_gat[1][128 * (kb % 4) : 128 * (kb % 4 + 1),
                                         (L // 2) * ph : (L // 2) * (ph + 1)],
                            )
                        for nth in range(L // 2 // NMM):
                            n0 = (L // 2) * ph + NMM * nth
                            pms = []
                            for o in range(2):
                                pc = slice(L * o + n0, L * o + n0 + NMM)
                                pm = psmm.tile([128, NMM], F32, tag="mm", name="pmm")
                                pms.append(pm)
                                for kb in range(4, 8):
                                    nc.tensor.matmul(
                                        pm[:],
                                        wsb[:, CH * kb + 128 * o :
                                            CH * kb + 128 * (o + 1)],
                                        yhs[:, (L // 2) * (kb - 4) + NMM * nth :
                                            (L // 2) * (kb - 4) + NMM * (nth + 1)],
                                        start=(kb == 4),
                                        stop=False,
                                    )
                                nc.tensor.matmul(
                                    pm[:],
                                    eyesb[:],
                                    part16[:, pc],
                                    start=False,
                                    stop=False,
                                )
                                nc.tensor.matmul(
                                    pm[:],
                                    eyesb[:],
                                    x16[o][:, n0 : n0 + NMM],
                                    start=False,
                                    stop=True,
                                )
                            ps_sum = psst.tile([1, NMM], F32, tag="sum", name="pssum")
                            ps_sq = psst.tile([1, NMM], F32, tag="sq", name="pssq")
                            for o in range(2):
                                zc = slice(L * o + n0, L * o + n0 + NMM)
                                nc.scalar.activation(
                                    z16[:, zc], pms[o][:], AF.Identity,
                                    bias=bsc[:, o : o + 1],
                                )
                                nc.tensor.matmul(
                                    ps_sum[:], ones16[:], z16[:, zc],
                                    start=(o == 0), stop=(o == 1),
                                    skip_group_check=True,
                                )
                                zq = tiny.tile([128, NMM], BF16, tag="z2", name="z2t")
                                nc.scalar.square(zq[:], z16[:, zc])
                                nc.tensor.matmul(
                                    ps_sq[:], ones16[:], zq[:],
                                    start=(o == 0), stop=(o == 1),
                                    skip_group_check=True,
                                )
                            sc_sum = tiny.tile([1, NMM], F32, tag="scsum", name="scsum")
                            sc_sq = tiny.tile([1, NMM], F32, tag="scsq", name="scsq")
                            nc.scalar.copy(sc_sum[:], ps_sum[:])
                            nc.scalar.copy(sc_sq[:], ps_sq[:])
                            nc.sync.dma_start(st_loc[0:1, n0 : n0 + NMM], sc_sum[:])
                            nc.sync.dma_start(st_loc[1:2, n0 : n0 + NMM], sc_sq[:])

                nc.gpsimd.collective_compute(
                    "AllReduce",
                    ALU.add,
                    replica_groups=GROUPS,
                    ins=[st_loc.opt()],
                    outs=[st_glb.opt()],
                )

                # LN scalar tail on [128, 32] layout (position t = 32p + f)
                with tc.tile_pool(name="lns", bufs=1) as lns:
                    s0 = lns.tile([128, 64], F32, name="s0")
                    mu32 = lns.tile([128, 32], F32, name="mu32")
                    msq = lns.tile([128, 32], F32, name="msq")
                    var32 = lns.tile([128, 32], F32, name="var32")
                    std32 = lns.tile([128, 32], F32, name="std32")
                    inv32 = lns.tile([128, 32], F32, name="inv32")
                    nms32 = lns.tile([128, 32], F32, name="nms32")
                    i16 = lns.tile([128, 32], BF16, name="i16")
                    n16 = lns.tile([128, 32], BF16, name="n16")
                    eps_t = lns.tile([128, 1], F32, name="eps_t")
                    nc.vector.memset(eps_t[:], LN_EPS)
                    stv = st_glb.rearrange("a (p f) -> a p f", p=128)
                    nc.sync.dma_start(s0[:, 0:32], stv[0])
                    nc.sync.dma_start(s0[:, 32:64], stv[1])
                    nc.scalar.mul(mu32[:], s0[:, 0:32], 1.0 / D)
                    nc.scalar.square(msq[:], mu32[:])
                    nc.vector.scalar_tensor_tensor(
                        var32[:], s0[:, 32:64], 1.0 / D, msq[:],
                        ALU.mult, ALU.subtract,
                    )
                    nc.scalar.activation(std32[:], var32[:], AF.Sqrt, bias=eps_t[:])
                    nc.vector.reciprocal_approx_fast(inv32[:], std32[:])
                    nc.vector.scalar_tensor_tensor(
                        nms32[:], mu32[:], -1.0, inv32[:], ALU.mult, ALU.mult
                    )
                    nc.vector.tensor_copy(i16[:], inv32[:])
                    nc.vector.tensor_copy(n16[:], nms32[:])
                    sfv = st_fin.rearrange("a (p f) -> a p f", p=128)
                    nc.sync.dma_start(sfv[0], i16[:])
                    nc.sync.dma_start(sfv[1], n16[:])

                ivec = mx.tile([1, L], BF16, name="ivec")
                nvec = mx.tile([1, L], BF16, name="nvec")
                nc.sync.dma_start(ivec[:], st_fin[0:1, :])
                nc.sync.dma_start(nvec[:], st_fin[1:2, :])

                # normalize: out = z*G + B2 with G/B2 via bf16 outer products
                osb = mx.tile([128, 2 * L], F32, name="osb")
                with tc.tile_pool(name="gbps", bufs=2, space="PSUM") as psgb:
                    for nt in range(L // NMM):
                        nn = slice(NMM * nt, NMM * (nt + 1))
                        for o in range(2):
                            oc = slice(128 * o, 128 * (o + 1))
                            zc = slice(L * o + NMM * nt, L * o + NMM * (nt + 1))
                            G = psgb.tile([128, NMM], F32, tag="G", name="G")
                            B2 = psgb.tile([128, NMM], F32, tag="B2", name="B2")
                            nc.tensor.matmul(G[:], grow[:, oc], ivec[:, nn])
                            nc.tensor.matmul(
                                B2[:], brow[:, oc], one_r[:],
                                start=True, stop=False,
                            )
                            nc.tensor.matmul(
                                B2[:], grow[:, oc], nvec[:, nn],
                                start=False, stop=True,
                            )
                            nc.vector.scalar_tensor_tensor(
                                osb[:, zc], z16[:, zc], 1.0, G[:],
                                ALU.mult, ALU.mult,
                            )
                            nc.vector.scalar_tensor_tensor(
                                osb[:, zc], osb[:, zc], 1.0, B2[:],
                                ALU.mult, ALU.add,
                            )
                            nc.sync.dma_start(
                                og[128 * o : 128 * (o + 1), nn],
                                osb[:, zc],
                            )

    nc.compile()
    return nc


def _get_program():
    key = "v5"
    if key not in _CACHE:
        _CACHE[key] = _build_program()
    return _CACHE[key]


def _make_in_maps(inputs):
    x = np.ascontiguousarray(np.asarray(inputs["x"], dtype=np.float32))
    h0 = np.asarray(inputs["h0"], dtype=np.float32)[:, 0, :]  # [D, FS]
    h1 = np.asarray(inputs["h1"], dtype=np.float32)[:, 0, :]
    w = np.asarray(inputs["w_mix"], dtype=np.float32)
    bm = np.asarray(inputs["b_mix"], dtype=np.float32).reshape(D, 1)
    gm = np.asarray(inputs["ln_gamma"], dtype=np.float32).reshape(1, D)
    bt = np.asarray(inputs["ln_beta"], dtype=np.float32).reshape(1, D)

    wT = np.ascontiguousarray(w.T)  # [c, o]
    # permute input-channel rows to the split-AllGather order:
    # new row r = h*512 + g*128 + j  <->  original channel c = g*256 + h*128 + j
    wTp = (
        wT.reshape(4, 2, 128, D)
        .transpose(1, 0, 2, 3)
        .reshape(D, D)
    )
    eye = np.eye(128, dtype=np.float32).astype(ml_dtypes.bfloat16)

    in_maps = []
    for c in range(NC):
        beta, gamma = c // 4, c % 4
        cs = slice(CH * gamma, CH * (gamma + 1))
        h0c = h0[cs].astype(ml_dtypes.bfloat16)
        h1c2 = (2.0 * h1[cs]).astype(ml_dtypes.bfloat16)
        d0m = np.zeros((2, FS, 128, 128), ml_dtypes.bfloat16)
        d1m = np.zeros((2, FS, 128, 128), ml_dtypes.bfloat16)
        for h in range(2):
            for k in range(FS):
                np.fill_diagonal(d0m[h, k], h0c[128 * h : 128 * (h + 1), k])
                np.fill_diagonal(d1m[h, k], h1c2[128 * h : 128 * (h + 1), k])
        in_maps.append(
            {
                "xs16": np.ascontiguousarray(x[beta, cs, :]).astype(
                    ml_dtypes.bfloat16),
                "h1s": np.ascontiguousarray(h1[cs]),
                "d0": d0m,
                "d1x2": d1m,
                "eye": eye,
                "wTs": np.ascontiguousarray(wTp[:, cs]).astype(ml_dtypes.bfloat16),
                "bmixs": np.ascontiguousarray(bm[cs]),
                "gams": np.ascontiguousarray(gm[:, cs]).astype(ml_dtypes.bfloat16),
                "bets": np.ascontiguousarray(bt[:, cs]).astype(ml_dtypes.bfloat16),
            }
        )
    return in_maps


def kernel(**inputs) -> np.ndarray:
    in_maps = _make_in_maps(inputs)
    nc = _get_program()
    res = run_bass_kernel_spmd(nc, in_maps, list(range(NC)))

    out = np.empty((B, D, L), dtype=np.float32)
    for c in range(NC):
        beta, gamma = c // 4, c % 4
        out[beta, CH * gamma : CH * (gamma + 1), :] = res.results[c]["og"]
    return out
